# revision 21
# baseline (speedup 1.0000x reference)
"""Trainium2 kernel for nn_CustomizedMoGPositionwiseFF (moe_routing).

Strategy (expert-parallel, per the sharding hint):
  - 32 (group, expert) FFN pairs are sharded across 8 NeuronCores (4 each).
  - Routing (group top-2 gate + per-group inner top-2 gate) is computed on
    host at call time; tokens are dispatched (gathered) per expert into the
    per-core shards -- data-dependent sharding, compiled into the NEFF.
  - Each core runs both FFN matmuls + relu for its 4 experts over the tokens
    routed to them, reading each expert weight exactly once (memory regime).
  - fp8 (e4m3) everywhere on-device with MatmulPerfMode.DoubleRow: each
    matmul contracts 2x128=256 at double pump rate, halving both PE time and
    HBM weight traffic vs bf16.  Scale bookkeeping: weights are shipped as
    64*W (fp8 normal range), z as-is (unit variance), h = relu(64*W1^T z) is
    cast to fp8 with no rescale (stays < 240), and the final u = 4096 * true
    is divided out in the host combine (free).
  - Host applies the cheap O(N*D) combine: iw/b2 scaling, scatter-add of the
    two expert contributions per (token, group), per-group post-layernorm,
    group top-2 mixture, and the outer residual.

The kernel output layout on device is u^T = (relu(z W1 + b1) W2)^T per
dispatched token, written as [D/128, 128, CT] so every DMA is dense.
"""

import os
import numpy as np

# Model dims (hardcoded per the contract; match the reference problem)
B, T, D, H = 2, 1024, 512, 2048
G, E, GK, EK = 4, 8, 2, 2
EPS = 1e-5
N = B * T
P = 128
DT = D // P    # 4 d-tiles
HT = H // P    # 16 h-tiles
DPAIRS = DT // 2   # DoubleRow k-pairs for layer 1
HPAIRS = HT // 2   # DoubleRow k-pairs for layer 2
NCORES = 8
SLOTS = (G * E) // NCORES  # 4 experts per core
CAP_GRAN = 16              # capacity granularity (tokens)
MAX_W = 512                # moving-dim chunk (PSUM bank = 512 f32; 2*W rows OK)
WSCALE = 64.0              # fp8 weight pre-scale (both layers)
USCALE = WSCALE * WSCALE   # device u = USCALE * true u

_nc_cache = {}
LAST_RESULTS = None       # test harness can inspect (BassKernelResults)


def _ensure_ntff_hook():
    """Register antenv.axon_hooks with the ctypes NTFF profile hook if the
    container's antenv package lacks it (mirrors trn_agent_boot.trn_boot).
    Makes trace=True work; degrades to hook=None when the .so is absent."""
    try:
        from antenv.axon_hooks import get_axon_ntff_profile_hook  # noqa: F401
        return
    except ImportError:
        pass
    import sys
    import types
    import contextlib
    import ctypes

    mod = types.ModuleType("antenv.axon_hooks")
    _state = {"hook": None}

    def set_axon_ntff_profile_hook(h):
        _state["hook"] = h

    def get_axon_ntff_profile_hook():
        return _state["hook"]

    mod.set_axon_ntff_profile_hook = set_axon_ntff_profile_hook
    mod.get_axon_ntff_profile_hook = get_axon_ntff_profile_hook

    so_path = "/opt/axon/libaxon_pjrt.so"
    hook = None
    if os.path.exists(so_path):
        try:
            lib = ctypes.CDLL(so_path)
            if hasattr(lib, "axon_start_nrt_profile"):
                lib.axon_start_nrt_profile.argtypes = [
                    ctypes.POINTER(ctypes.c_int64), ctypes.c_size_t]
                lib.axon_start_nrt_profile.restype = ctypes.c_int64
                lib.axon_stop_nrt_profile.argtypes = [ctypes.c_char_p]
                lib.axon_stop_nrt_profile.restype = ctypes.c_int64

                @contextlib.contextmanager
                def _hook(output_dir, device_ids):
                    import jax
                    jax.devices()
                    if device_ids:
                        ids = (ctypes.c_int64 * len(device_ids))(*device_ids)
                        rc = lib.axon_start_nrt_profile(ids, len(device_ids))
                    else:
                        rc = lib.axon_start_nrt_profile(None, 0)
                    if rc != 0:
                        raise RuntimeError(f"axon_start_nrt_profile rc={rc}")
                    try:
                        yield
                    finally:
                        n = lib.axon_stop_nrt_profile(str(output_dir).encode())
                        print(f"ntff profile: {n} file(s) -> {output_dir}")

                hook = _hook
        except Exception:
            hook = None
    _state["hook"] = hook
    import antenv
    sys.modules["antenv.axon_hooks"] = mod
    antenv.axon_hooks = mod


def _round_up(x, m):
    return ((x + m - 1) // m) * m


def _chunks(C):
    """Split capacity C into even moving-dim chunks of at most MAX_W."""
    n = (C + MAX_W - 1) // MAX_W
    w = _round_up((C + n - 1) // n, 8)
    out = [w] * (n - 1) + [C - (n - 1) * w]
    assert all(0 < c <= MAX_W for c in out) and sum(out) == C, (C, out)
    return out


def _routing(inp, ln_g, ln_b, wg_group, wg_inner):
    """Replicate the reference gating bit-for-bit on jax-cpu.

    Returns gi [N,GK] group ids, gsc [N,GK] group softmax, z [N,D] f32,
    eis/escs: per-group inner top-k ids/softmax ([N,EK] each).
    """
    import jax
    import jax.numpy as jnp

    cpu = jax.devices("cpu")[0]
    with jax.default_device(cpu):
        x = jnp.asarray(np.asarray(inp, np.float32)).reshape(-1, D)
        gl = x @ jnp.asarray(np.asarray(wg_group, np.float32))
        gv, gi = jax.lax.top_k(gl, GK)
        gsc = jax.nn.softmax(gv, axis=-1)
        m = jnp.mean(x, axis=-1, keepdims=True)
        xc = x - m
        v = jnp.mean(xc * xc, axis=-1, keepdims=True)
        z = xc * jax.lax.rsqrt(v + EPS) * jnp.asarray(np.asarray(ln_g, np.float32)) \
            + jnp.asarray(np.asarray(ln_b, np.float32))
        wgi = jnp.asarray(np.asarray(wg_inner, np.float32))
        eis, escs = [], []
        for g in range(G):
            l = z @ wgi[g]
            ev, ei = jax.lax.top_k(l, EK)
            esc = jax.nn.softmax(ev, axis=-1)
            eis.append(np.asarray(ei))
            escs.append(np.asarray(esc))
    return np.asarray(gi), np.asarray(gsc), np.asarray(z), eis, escs


def _build_nc(Cs, has_b1=False):
    """Build the SPMD Bass program for per-slot capacities Cs (uniform across cores)."""
    import concourse.bass as bass
    import concourse.bacc as bacc
    import concourse.tile as tile
    from concourse import mybir

    f32 = mybir.dt.float32
    bf16 = mybir.dt.bfloat16
    fp8 = mybir.dt.float8e4
    DR = mybir.MatmulPerfMode.DoubleRow
    Relu = mybir.ActivationFunctionType.Relu

    CT = int(sum(Cs))
    offs = np.concatenate([[0], np.cumsum(Cs)]).astype(int)

    nc = bacc.Bacc("TRN2", target_bir_lowering=False)
    # all DRAM layouts are partition-major [128, free] so every DMA is 128
    # contiguous lines (max-size descriptors, cheap HWDGE issue)
    zt_d = nc.declare_dram_parameter("zt", [P, DT * CT], fp8, isOutput=False)
    w1_d = nc.declare_dram_parameter("w1", [SLOTS, P, DT * H], fp8, isOutput=False)
    w2_d = nc.declare_dram_parameter("w2", [SLOTS, P, HT * D], fp8, isOutput=False)
    b1_d = nc.declare_dram_parameter("b1", [P, SLOTS * HT], f32, isOutput=False)
    u_d = nc.declare_dram_parameter("u", [P, DT * CT], bf16, isOutput=True)

    with tile.TileContext(nc) as tc:
        with tc.tile_pool(name="consts", bufs=1) as consts, \
             tc.tile_pool(name="hpool", bufs=2) as hpool, \
             tc.tile_pool(name="hpsum", bufs=3, space="PSUM") as hpsum, \
             tc.tile_pool(name="upsum", bufs=2, space="PSUM") as upsum, \
             tc.tile_pool(name="usb", bufs=4) as usb:

            zt_sb = consts.tile([P, DT * CT], fp8, tag="zt")
            b1_sb = consts.tile([P, SLOTS * HT], f32, tag="b1")
            zero_sb = consts.tile([P, MAX_W], f32, tag="zero")
            warm_sb = consts.tile([P, 2], f32, tag="warm")
            nc.vector.memset(zero_sb[:, :], 0.0)
            w1_sb, w2_sb = [], []
            for s in range(SLOTS):
                w1_sb.append(consts.tile([P, DT * H], fp8, tag=f"w1_{s}", name=f"w1s_{s}"))
                w2_sb.append(consts.tile([P, HT * D], fp8, tag=f"w2_{s}", name=f"w2s_{s}"))

            # ---- resident loads.  Each DMA trigger costs ~650ns of ring
            # issue time, so the count and ORDER on the two HWDGE rings
            # (Sync / Scalar) is what sets the time-to-first-matmul and the
            # per-slot weight arrival.  w1 DRAM layout is (hhalf, dt, hcol)
            # and w2 is (dt, ht, dcol), so the first-needed weights of each
            # layer are contiguous prefixes.
            HH = DT * (H // 2)          # w1 bytes per h-half
            DH = HT * (D // 2)          # w2 bytes per dt-half
            # sync ring: zt (dpair halves), w2[0] (dt halves), rest of w2
            nc.sync.dma_start(zt_sb[:, :2 * CT], zt_d[:, :2 * CT])
            nc.sync.dma_start(zt_sb[:, 2 * CT:], zt_d[:, 2 * CT:])
            nc.sync.dma_start(w2_sb[0][:, :DH], w2_d[0][:, :DH])
            nc.sync.dma_start(w2_sb[0][:, DH:], w2_d[0][:, DH:])
            if has_b1:
                nc.sync.dma_start(b1_sb[:, :], b1_d[:, :])
            for s in range(1, SLOTS):
                nc.sync.dma_start(w2_sb[s][:, :], w2_d[s][:, :])
            # scalar ring: all w1, slot 0 split by h-half
            nc.scalar.dma_start(w1_sb[0][:, :HH], w1_d[0][:, :HH])
            nc.scalar.dma_start(w1_sb[0][:, HH:], w1_d[0][:, HH:])
            for s in range(1, SLOTS):
                nc.scalar.dma_start(w1_sb[s][:, :], w1_d[s][:, :])
            # preload the ACT table containing Relu/Copy while DMAs stream
            # (after the triggers: the table load must not delay them)
            nc.scalar.activation(
                warm_sb[:, 1:2], warm_sb[:, 0:1],
                mybir.ActivationFunctionType.Relu)

            ztr = zt_sb.rearrange("p (dt c) -> p dt c", dt=DT)
            ur_d = u_d.rearrange("p (dt c) -> p dt c", dt=DT)

            # ---- compute
            for s in range(SLOTS):
                # w1 free layout is (hhalf, dt, hcol)
                w1r = w1_sb[s].rearrange(
                    "p (hh dt hc) -> p hh dt hc", hh=2, dt=DT)
                # w2 free layout is (dt, ht, dcol)
                w2r = w2_sb[s].rearrange(
                    "p (dt ht c) -> p dt ht c", dt=DT, ht=HT)
                c0 = 0
                for W in _chunks(int(Cs[s])):
                    off = int(offs[s]) + c0
                    c0 += W
                    h_sb = hpool.tile([P, HT * W], fp8, tag="h")
                    # layer 1 in groups of 2 ht, j-ordered inside the group
                    # so the first matmuls only need the dpair-0 data
                    for hg in range(0, HT, 2):
                        phs = [hpsum.tile([P, W], f32, tag=f"ph{t}",
                                          name=f"ph_{s}_{c0}_{hg}_{t}")
                               for t in range(2)]
                        for j in range(DPAIRS):
                            for t in range(2):
                                ht = hg + t
                                hh, hc = ht // 8, (ht % 8) * P
                                nc.tensor.matmul(
                                    phs[t][:, :],
                                    w1r[:, hh, 2 * j:2 * j + 2, hc:hc + P],
                                    ztr[:, 2 * j:2 * j + 2, off:off + W],
                                    start=(j == 0),
                                    stop=(j == DPAIRS - 1),
                                    perf_mode=DR,
                                )
                        for t in range(2):
                            ht = hg + t
                            if has_b1:
                                # general path: ACT relu with per-partition
                                # bias (bias pre-scaled by WSCALE on host)
                                nc.scalar.activation(
                                    h_sb[:, ht * W:(ht + 1) * W], phs[t][:, :],
                                    Relu,
                                    bias=b1_sb[:, s * HT + ht: s * HT + ht + 1],
                                )
                            elif ht % 2 == 0:
                                # psum->sbuf post-ops run at ~1/3 DVE peak:
                                # split across DVE and ACT to stay off the
                                # critical path (PE is the roofline engine)
                                nc.vector.tensor_max(
                                    h_sb[:, ht * W:(ht + 1) * W], phs[t][:, :],
                                    zero_sb[:, :W],
                                )
                            else:
                                nc.scalar.activation(
                                    h_sb[:, ht * W:(ht + 1) * W], phs[t][:, :],
                                    Relu,
                                )
                    hr = h_sb.rearrange("p (ht w) -> p ht w", ht=HT)
                    # layer 2: u^T[dt] = sum_ht W2[ht,dt]^T h^T[ht]
                    u_sb = usb.tile([P, DT * W], bf16, tag="u")
                    for dt in range(DT):
                        pu = upsum.tile([P, W], f32, tag="pu")
                        for j in range(HPAIRS):
                            nc.tensor.matmul(
                                pu[:, :],
                                w2r[:, dt, 2 * j:2 * j + 2, :],
                                hr[:, 2 * j:2 * j + 2, :],
                                start=(j == 0),
                                stop=(j == HPAIRS - 1),
                                perf_mode=DR,
                            )
                        if dt % 2 == 0:
                            nc.vector.tensor_copy(u_sb[:, dt * W:(dt + 1) * W], pu[:, :])
                        else:
                            nc.scalar.activation(
                                u_sb[:, dt * W:(dt + 1) * W], pu[:, :],
                                mybir.ActivationFunctionType.Copy,
                            )
                    # one output DMA per slot-chunk on the sync HWDGE ring
                    # (free after the weight loads; SWDGE is far slower, and
                    # many small transfers back the ring up at the tail)
                    nc.sync.dma_start(
                        ur_d[:, :, off:off + W],
                        u_sb.rearrange("p (d c) -> p d c", d=DT),
                    )
    nc.compile()
    return nc


def _get_nc(Cs, has_b1):
    key = (tuple(int(c) for c in Cs), bool(has_b1))
    if key not in _nc_cache:
        _nc_cache[key] = _build_nc(key[0], key[1])
    return _nc_cache[key]


def kernel(inp, ln_g, ln_b, wg_group, wg_inner, W1, b1, W2, b2, gln_g, gln_b):
    global LAST_RESULTS
    import jax
    import jax.numpy as jnp
    import ml_dtypes

    inp = np.asarray(inp)
    in_dtype = inp.dtype
    fp8 = ml_dtypes.float8_e4m3

    # ---- 1. routing on host (bit-exact replica of the reference gates)
    gi, gsc, z, eis, escs = _routing(inp, ln_g, ln_b, wg_group, wg_inner)

    # token lists per (g, e)
    tok_lists, scale_lists = {}, {}
    for g in range(G):
        in_g = (gi == g).any(axis=1)
        S_g = np.nonzero(in_g)[0]
        ei, esc = eis[g], escs[g]
        for e in range(E):
            sel = ei[S_g] == e           # [|S_g|, EK]
            has = sel.any(axis=1)
            toks = S_g[has]
            w = (esc[S_g] * sel).sum(axis=1)[has]
            tok_lists[(g, e)] = toks
            scale_lists[(g, e)] = w.astype(np.float32)

    # ---- 2. balanced assignment of the 32 pairs to (core, slot)
    pairs = [(g, e) for g in range(G) for e in range(E)]
    pairs.sort(key=lambda p: -len(tok_lists[p]))
    assign = {}           # (core, slot) -> (g, e)
    Cs = []
    for s in range(SLOTS):
        rank = pairs[s * NCORES:(s + 1) * NCORES]
        Cs.append(max(CAP_GRAN, _round_up(max(len(tok_lists[p]) for p in rank), CAP_GRAN)))
        for c, p in enumerate(rank):
            assign[(c, s)] = p
    CT = int(sum(Cs))
    offs = np.concatenate([[0], np.cumsum(Cs)]).astype(int)

    # ---- 3. build per-core input maps (fp8, weights pre-scaled by WSCALE)
    W1n = np.asarray(W1, np.float32) * WSCALE
    W2n = np.asarray(W2, np.float32) * WSCALE
    b1n = np.asarray(b1, np.float32)
    b2n = np.asarray(b2, np.float32)
    z_fp8 = z.astype(fp8)

    in_maps = []
    for c in range(NCORES):
        # partition-major device layouts (see _build_nc)
        zt_np = np.zeros((P, DT * CT), fp8)
        w1_np = np.empty((SLOTS, P, DT * H), fp8)
        w2_np = np.empty((SLOTS, P, HT * D), fp8)
        b1_np = np.empty((P, SLOTS * HT), np.float32)
        zt_v = zt_np.reshape(P, DT, CT)
        b1_v = b1_np.reshape(P, SLOTS, HT)
        for s in range(SLOTS):
            g, e = assign[(c, s)]
            toks = tok_lists[(g, e)]
            n = len(toks)
            off = offs[s]
            # z^T tile (dt, p, c) -> [p, dt, c]
            zt_v[:, :, off:off + n] = z_fp8[toks].T.reshape(DT, P, n).transpose(1, 0, 2)
            # device w1 free layout is (hhalf, dt, hcol)
            w1_np[s] = (
                W1n[g, e].astype(fp8).reshape(DT, P, 2, H // 2)
                .transpose(1, 2, 0, 3).reshape(P, DT * H)
            )
            # device w2 free layout is (dt, ht, dcol)
            w2_np[s] = (
                W2n[g, e].astype(fp8).reshape(HT, P, DT, P)
                .transpose(1, 2, 0, 3).reshape(P, HT * D)
            )
            # bias scaled to match the WSCALE'd layer-1 products
            b1_v[:, s, :] = (b1n[g, e] * WSCALE).reshape(HT, P).T
        in_maps.append({"zt": zt_np, "w1": w1_np, "w2": w2_np, "b1": b1_np})

    # ---- 4. compile + run on the 8 NeuronCores
    _ensure_ntff_hook()
    from concourse.bass_utils import run_bass_kernel_spmd

    nc = _get_nc(Cs, has_b1=bool(np.any(b1n)))
    res = run_bass_kernel_spmd(
        nc, in_maps, core_ids=list(range(NCORES)),
        trace=bool(int(os.environ.get("KERNEL_TRACE", "0"))),
    )
    LAST_RESULTS = res

    # ---- 5. host combine (u comes back scaled by USCALE; fold 1/USCALE in)
    moe = np.zeros((G, N, D), np.float32)
    for c in range(NCORES):
        # u layout [p, dt*CT + c] -> u^T[d, c] -> [CT, D]
        u = (
            np.asarray(res.results[c]["u"], np.float32)
            .reshape(P, DT, CT).transpose(1, 0, 2).reshape(D, CT).T
        )
        for s in range(SLOTS):
            g, e = assign[(c, s)]
            toks = tok_lists[(g, e)]
            n = len(toks)
            w = scale_lists[(g, e)]
            contrib = u[offs[s]:offs[s] + n] * (w / USCALE)[:, None] \
                + w[:, None] * b2n[g, e][None, :]
            np.add.at(moe[g], toks, contrib)

    cpu = jax.devices("cpu")[0]
    with jax.default_device(cpu):
        zj = jnp.asarray(z)
        gi_j = jnp.asarray(gi)
        gsc_j = jnp.asarray(gsc)
        gw_dense = jnp.sum(
            jax.nn.one_hot(gi_j, G, dtype=jnp.float32) * gsc_j[..., None], axis=-2
        )  # [N, G]
        out = jnp.zeros((N, D), jnp.float32)
        gg = jnp.asarray(np.asarray(gln_g, np.float32))
        gb = jnp.asarray(np.asarray(gln_b, np.float32))
        for g in range(G):
            t = zj + jnp.asarray(moe[g])
            m = jnp.mean(t, axis=-1, keepdims=True)
            tc_ = t - m
            v = jnp.mean(tc_ * tc_, axis=-1, keepdims=True)
            y = tc_ * jax.lax.rsqrt(v + EPS) * gg[g] + gb[g]
            out = out + gw_dense[:, g:g + 1] * y
        result = np.asarray(out).reshape(B, T, D) + np.asarray(inp, np.float32)

    return result.astype(in_dtype)


# revision 24
# speedup vs baseline: 1.1224x; 1.1224x over previous
"""Trainium2 kernel for nn_CustomizedMoGPositionwiseFF (moe_routing).

Strategy (expert-parallel, per the sharding hint):
  - 32 (group, expert) FFN pairs are sharded across 8 NeuronCores (4 each).
  - Routing (group top-2 gate + per-group inner top-2 gate) is computed on
    host at call time; tokens are dispatched (gathered) per expert into the
    per-core shards -- data-dependent sharding, compiled into the NEFF.
  - Each core runs both FFN matmuls + relu for its 4 experts over the tokens
    routed to them, reading each expert weight exactly once (memory regime).
  - fp8 (e4m3) everywhere on-device with MatmulPerfMode.DoubleRow: each
    matmul contracts 2x128=256 at double pump rate, halving both PE time and
    HBM weight traffic vs bf16.  Scale bookkeeping: weights are shipped as
    64*W (fp8 normal range), z as-is (unit variance), h = relu(64*W1^T z) is
    cast to fp8 with no rescale (stays < 240), and the final u = 4096 * true
    is divided out in the host combine (free).
  - Host applies the cheap O(N*D) combine: iw/b2 scaling, scatter-add of the
    two expert contributions per (token, group), per-group post-layernorm,
    group top-2 mixture, and the outer residual.

The kernel output layout on device is u^T = (relu(z W1 + b1) W2)^T per
dispatched token, written as [D/128, 128, CT] so every DMA is dense.
"""

import os
import numpy as np

# Model dims (hardcoded per the contract; match the reference problem)
B, T, D, H = 2, 1024, 512, 2048
G, E, GK, EK = 4, 8, 2, 2
EPS = 1e-5
N = B * T
P = 128
DT = D // P    # 4 d-tiles
HT = H // P    # 16 h-tiles
DPAIRS = DT // 2   # DoubleRow k-pairs for layer 1
HPAIRS = HT // 2   # DoubleRow k-pairs for layer 2
NCORES = 8
SLOTS = (G * E) // NCORES  # 4 experts per core
CAP_GRAN = 16              # capacity granularity (tokens)
MAX_W = 512                # moving-dim chunk (PSUM bank = 512 f32; 2*W rows OK)
WSCALE = 64.0              # fp8 weight pre-scale (both layers)
USCALE = WSCALE * WSCALE   # device u = USCALE * true u

_nc_cache = {}
LAST_RESULTS = None       # test harness can inspect (BassKernelResults)


def _ensure_ntff_hook():
    """Register antenv.axon_hooks with the ctypes NTFF profile hook if the
    container's antenv package lacks it (mirrors trn_agent_boot.trn_boot).
    Makes trace=True work; degrades to hook=None when the .so is absent."""
    try:
        from antenv.axon_hooks import get_axon_ntff_profile_hook  # noqa: F401
        return
    except ImportError:
        pass
    import sys
    import types
    import contextlib
    import ctypes

    mod = types.ModuleType("antenv.axon_hooks")
    _state = {"hook": None}

    def set_axon_ntff_profile_hook(h):
        _state["hook"] = h

    def get_axon_ntff_profile_hook():
        return _state["hook"]

    mod.set_axon_ntff_profile_hook = set_axon_ntff_profile_hook
    mod.get_axon_ntff_profile_hook = get_axon_ntff_profile_hook

    so_path = "/opt/axon/libaxon_pjrt.so"
    hook = None
    if os.path.exists(so_path):
        try:
            lib = ctypes.CDLL(so_path)
            if hasattr(lib, "axon_start_nrt_profile"):
                lib.axon_start_nrt_profile.argtypes = [
                    ctypes.POINTER(ctypes.c_int64), ctypes.c_size_t]
                lib.axon_start_nrt_profile.restype = ctypes.c_int64
                lib.axon_stop_nrt_profile.argtypes = [ctypes.c_char_p]
                lib.axon_stop_nrt_profile.restype = ctypes.c_int64

                @contextlib.contextmanager
                def _hook(output_dir, device_ids):
                    import jax
                    jax.devices()
                    if device_ids:
                        ids = (ctypes.c_int64 * len(device_ids))(*device_ids)
                        rc = lib.axon_start_nrt_profile(ids, len(device_ids))
                    else:
                        rc = lib.axon_start_nrt_profile(None, 0)
                    if rc != 0:
                        raise RuntimeError(f"axon_start_nrt_profile rc={rc}")
                    try:
                        yield
                    finally:
                        n = lib.axon_stop_nrt_profile(str(output_dir).encode())
                        print(f"ntff profile: {n} file(s) -> {output_dir}")

                hook = _hook
        except Exception:
            hook = None
    _state["hook"] = hook
    import antenv
    sys.modules["antenv.axon_hooks"] = mod
    antenv.axon_hooks = mod


def _round_up(x, m):
    return ((x + m - 1) // m) * m


def _chunks(C):
    """Split capacity C into even moving-dim chunks of at most MAX_W."""
    n = (C + MAX_W - 1) // MAX_W
    w = _round_up((C + n - 1) // n, 8)
    out = [w] * (n - 1) + [C - (n - 1) * w]
    assert all(0 < c <= MAX_W for c in out) and sum(out) == C, (C, out)
    return out


def _routing(inp, ln_g, ln_b, wg_group, wg_inner):
    """Replicate the reference gating bit-for-bit on jax-cpu.

    Returns gi [N,GK] group ids, gsc [N,GK] group softmax, z [N,D] f32,
    eis/escs: per-group inner top-k ids/softmax ([N,EK] each).
    """
    import jax
    import jax.numpy as jnp

    cpu = jax.devices("cpu")[0]
    with jax.default_device(cpu):
        x = jnp.asarray(np.asarray(inp, np.float32)).reshape(-1, D)
        gl = x @ jnp.asarray(np.asarray(wg_group, np.float32))
        gv, gi = jax.lax.top_k(gl, GK)
        gsc = jax.nn.softmax(gv, axis=-1)
        m = jnp.mean(x, axis=-1, keepdims=True)
        xc = x - m
        v = jnp.mean(xc * xc, axis=-1, keepdims=True)
        z = xc * jax.lax.rsqrt(v + EPS) * jnp.asarray(np.asarray(ln_g, np.float32)) \
            + jnp.asarray(np.asarray(ln_b, np.float32))
        wgi = jnp.asarray(np.asarray(wg_inner, np.float32))
        eis, escs = [], []
        for g in range(G):
            l = z @ wgi[g]
            ev, ei = jax.lax.top_k(l, EK)
            esc = jax.nn.softmax(ev, axis=-1)
            eis.append(np.asarray(ei))
            escs.append(np.asarray(esc))
    return np.asarray(gi), np.asarray(gsc), np.asarray(z), eis, escs


def _build_nc(Cs, has_b1=False):
    """Build the SPMD Bass program for per-slot capacities Cs (uniform across cores)."""
    import concourse.bass as bass
    import concourse.bacc as bacc
    import concourse.tile as tile
    from concourse import mybir

    f32 = mybir.dt.float32
    bf16 = mybir.dt.bfloat16
    fp8 = mybir.dt.float8e4
    DR = mybir.MatmulPerfMode.DoubleRow
    Relu = mybir.ActivationFunctionType.Relu

    CT = int(sum(Cs))
    offs = np.concatenate([[0], np.cumsum(Cs)]).astype(int)

    nc = bacc.Bacc("TRN2", target_bir_lowering=False)
    # all DRAM layouts are partition-major [128, free] so every DMA is 128
    # contiguous lines (max-size descriptors, cheap HWDGE issue)
    zt_d = nc.declare_dram_parameter("zt", [P, DT * CT], fp8, isOutput=False)
    w1_d = nc.declare_dram_parameter("w1", [SLOTS, P, DT * H], fp8, isOutput=False)
    w2_d = nc.declare_dram_parameter("w2", [SLOTS, P, HT * D], fp8, isOutput=False)
    b1_d = nc.declare_dram_parameter("b1", [P, SLOTS * HT], f32, isOutput=False)
    u_d = nc.declare_dram_parameter("u", [P, DT * CT], bf16, isOutput=True)

    with tile.TileContext(nc) as tc:
        with tc.tile_pool(name="consts", bufs=1) as consts, \
             tc.tile_pool(name="hpool", bufs=2) as hpool, \
             tc.tile_pool(name="hpsum", bufs=3, space="PSUM") as hpsum, \
             tc.tile_pool(name="upsum", bufs=2, space="PSUM") as upsum, \
             tc.tile_pool(name="usb", bufs=4) as usb:

            zt_sb = consts.tile([P, DT * CT], fp8, tag="zt")
            b1_sb = consts.tile([P, SLOTS * HT], f32, tag="b1")
            zero_sb = consts.tile([P, MAX_W], f32, tag="zero")
            warm_sb = consts.tile([P, 2], f32, tag="warm")
            wfp8_sb = consts.tile([P, 2 * MAX_W], fp8, tag="wfp8")
            nc.vector.memset(zero_sb[:, :], 0.0)
            nc.vector.memset(wfp8_sb[:, :], 0.0)
            w1_sb, w2_sb = [], []
            for s in range(SLOTS):
                w1_sb.append(consts.tile([P, DT * H], fp8, tag=f"w1_{s}", name=f"w1s_{s}"))
                w2_sb.append(consts.tile([P, HT * D], fp8, tag=f"w2_{s}", name=f"w2s_{s}"))

            # ---- resident loads.  Each DMA trigger costs ~650ns of ring
            # issue time, so the count and ORDER on the two HWDGE rings
            # (Sync / Scalar) is what sets the time-to-first-matmul and the
            # per-slot weight arrival.  w1 DRAM layout is (hhalf, dt, hcol)
            # and w2 is (dt, ht, dcol), so the first-needed weights of each
            # layer are contiguous prefixes.
            HH = DT * (H // 2)          # w1 bytes per h-half
            DH = HT * (D // 2)          # w2 bytes per dt-half
            # sync ring: zt (dpair halves), w2[0] (dt halves), rest of w2
            nc.sync.dma_start(zt_sb[:, :2 * CT], zt_d[:, :2 * CT])
            nc.sync.dma_start(zt_sb[:, 2 * CT:], zt_d[:, 2 * CT:])
            nc.sync.dma_start(w2_sb[0][:, :DH], w2_d[0][:, :DH])
            nc.sync.dma_start(w2_sb[0][:, DH:], w2_d[0][:, DH:])
            if has_b1:
                nc.sync.dma_start(b1_sb[:, :], b1_d[:, :])
            for s in range(1, SLOTS):
                nc.sync.dma_start(w2_sb[s][:, :], w2_d[s][:, :])
            # scalar ring: all w1, slot 0 split by h-half
            nc.scalar.dma_start(w1_sb[0][:, :HH], w1_d[0][:, :HH])
            nc.scalar.dma_start(w1_sb[0][:, HH:], w1_d[0][:, HH:])
            for s in range(1, SLOTS):
                nc.scalar.dma_start(w1_sb[s][:, :], w1_d[s][:, :])
            # preload the ACT table containing Relu/Copy while DMAs stream
            # (after the triggers: the table load must not delay them)
            nc.scalar.activation(
                warm_sb[:, 1:2], warm_sb[:, 0:1],
                mybir.ActivationFunctionType.Relu)

            # ---- PE warmup.  The PE clock p-state ramps over ~3us of
            # continuous execution; the engine would otherwise sit idle
            # until the first weights land (~7us) and pay the ramp on real
            # work.  Run throwaway DoubleRow matmuls on a zeroed tile to
            # arrive at the first real matmul already at full clock.
            wr = wfp8_sb.rearrange("p (two n) -> p two n", two=2)
            warm_ps = upsum.tile([P, MAX_W], f32, tag="pu", name="warm_pu")
            for _ in range(20):
                nc.tensor.matmul(
                    warm_ps[:, :], wr[:, :, :P], wr[:, :, :],
                    start=True, stop=True, perf_mode=DR,
                )

            ztr = zt_sb.rearrange("p (dt c) -> p dt c", dt=DT)
            ur_d = u_d.rearrange("p (dt c) -> p dt c", dt=DT)

            # ---- compute
            for s in range(SLOTS):
                # w1 free layout is (hhalf, dt, hcol)
                w1r = w1_sb[s].rearrange(
                    "p (hh dt hc) -> p hh dt hc", hh=2, dt=DT)
                # w2 free layout is (dt, ht, dcol)
                w2r = w2_sb[s].rearrange(
                    "p (dt ht c) -> p dt ht c", dt=DT, ht=HT)
                c0 = 0
                for W in _chunks(int(Cs[s])):
                    off = int(offs[s]) + c0
                    c0 += W
                    h_sb = hpool.tile([P, HT * W], fp8, tag="h")
                    # layer 1 in groups of 2 ht, j-ordered inside the group
                    # so the first matmuls only need the dpair-0 data
                    for hg in range(0, HT, 2):
                        phs = [hpsum.tile([P, W], f32, tag=f"ph{t}",
                                          name=f"ph_{s}_{c0}_{hg}_{t}")
                               for t in range(2)]
                        for j in range(DPAIRS):
                            for t in range(2):
                                ht = hg + t
                                hh, hc = ht // 8, (ht % 8) * P
                                nc.tensor.matmul(
                                    phs[t][:, :],
                                    w1r[:, hh, 2 * j:2 * j + 2, hc:hc + P],
                                    ztr[:, 2 * j:2 * j + 2, off:off + W],
                                    start=(j == 0),
                                    stop=(j == DPAIRS - 1),
                                    perf_mode=DR,
                                )
                        for t in range(2):
                            ht = hg + t
                            if has_b1:
                                # general path: ACT relu with per-partition
                                # bias (bias pre-scaled by WSCALE on host)
                                nc.scalar.activation(
                                    h_sb[:, ht * W:(ht + 1) * W], phs[t][:, :],
                                    Relu,
                                    bias=b1_sb[:, s * HT + ht: s * HT + ht + 1],
                                )
                            elif ht % 2 == 0:
                                # psum->sbuf post-ops run at ~1/3 DVE peak:
                                # split across DVE and ACT to stay off the
                                # critical path (PE is the roofline engine)
                                nc.vector.tensor_max(
                                    h_sb[:, ht * W:(ht + 1) * W], phs[t][:, :],
                                    zero_sb[:, :W],
                                )
                            else:
                                nc.scalar.activation(
                                    h_sb[:, ht * W:(ht + 1) * W], phs[t][:, :],
                                    Relu,
                                )
                    hr = h_sb.rearrange("p (ht w) -> p ht w", ht=HT)
                    # layer 2: u^T[dt] = sum_ht W2[ht,dt]^T h^T[ht]
                    u_sb = usb.tile([P, DT * W], bf16, tag="u")
                    for dt in range(DT):
                        pu = upsum.tile([P, W], f32, tag="pu")
                        for j in range(HPAIRS):
                            nc.tensor.matmul(
                                pu[:, :],
                                w2r[:, dt, 2 * j:2 * j + 2, :],
                                hr[:, 2 * j:2 * j + 2, :],
                                start=(j == 0),
                                stop=(j == HPAIRS - 1),
                                perf_mode=DR,
                            )
                        nc.vector.tensor_copy(u_sb[:, dt * W:(dt + 1) * W], pu[:, :])
                        # per-dt output DMA on the sync HWDGE ring (free
                        # after the weight loads; SWDGE is far slower, and
                        # 3D triggers cost 4x the ring issue time)
                        nc.sync.dma_start(
                            ur_d[:, dt, off:off + W],
                            u_sb[:, dt * W:(dt + 1) * W],
                        )
    nc.compile()
    return nc


def _get_nc(Cs, has_b1):
    key = (tuple(int(c) for c in Cs), bool(has_b1))
    if key not in _nc_cache:
        _nc_cache[key] = _build_nc(key[0], key[1])
    return _nc_cache[key]


def kernel(inp, ln_g, ln_b, wg_group, wg_inner, W1, b1, W2, b2, gln_g, gln_b):
    global LAST_RESULTS
    import jax
    import jax.numpy as jnp
    import ml_dtypes

    inp = np.asarray(inp)
    in_dtype = inp.dtype
    fp8 = ml_dtypes.float8_e4m3

    # ---- 1. routing on host (bit-exact replica of the reference gates)
    gi, gsc, z, eis, escs = _routing(inp, ln_g, ln_b, wg_group, wg_inner)

    # token lists per (g, e)
    tok_lists, scale_lists = {}, {}
    for g in range(G):
        in_g = (gi == g).any(axis=1)
        S_g = np.nonzero(in_g)[0]
        ei, esc = eis[g], escs[g]
        for e in range(E):
            sel = ei[S_g] == e           # [|S_g|, EK]
            has = sel.any(axis=1)
            toks = S_g[has]
            w = (esc[S_g] * sel).sum(axis=1)[has]
            tok_lists[(g, e)] = toks
            scale_lists[(g, e)] = w.astype(np.float32)

    # ---- 2. balanced assignment of the 32 pairs to (core, slot)
    pairs = [(g, e) for g in range(G) for e in range(E)]
    pairs.sort(key=lambda p: -len(tok_lists[p]))
    assign = {}           # (core, slot) -> (g, e)
    Cs = []
    for s in range(SLOTS):
        rank = pairs[s * NCORES:(s + 1) * NCORES]
        Cs.append(max(CAP_GRAN, _round_up(max(len(tok_lists[p]) for p in rank), CAP_GRAN)))
        for c, p in enumerate(rank):
            assign[(c, s)] = p
    CT = int(sum(Cs))
    offs = np.concatenate([[0], np.cumsum(Cs)]).astype(int)

    # ---- 3. build per-core input maps (fp8, weights pre-scaled by WSCALE)
    W1n = np.asarray(W1, np.float32) * WSCALE
    W2n = np.asarray(W2, np.float32) * WSCALE
    b1n = np.asarray(b1, np.float32)
    b2n = np.asarray(b2, np.float32)
    z_fp8 = z.astype(fp8)

    in_maps = []
    for c in range(NCORES):
        # partition-major device layouts (see _build_nc)
        zt_np = np.zeros((P, DT * CT), fp8)
        w1_np = np.empty((SLOTS, P, DT * H), fp8)
        w2_np = np.empty((SLOTS, P, HT * D), fp8)
        b1_np = np.empty((P, SLOTS * HT), np.float32)
        zt_v = zt_np.reshape(P, DT, CT)
        b1_v = b1_np.reshape(P, SLOTS, HT)
        for s in range(SLOTS):
            g, e = assign[(c, s)]
            toks = tok_lists[(g, e)]
            n = len(toks)
            off = offs[s]
            # z^T tile (dt, p, c) -> [p, dt, c]
            zt_v[:, :, off:off + n] = z_fp8[toks].T.reshape(DT, P, n).transpose(1, 0, 2)
            # device w1 free layout is (hhalf, dt, hcol)
            w1_np[s] = (
                W1n[g, e].astype(fp8).reshape(DT, P, 2, H // 2)
                .transpose(1, 2, 0, 3).reshape(P, DT * H)
            )
            # device w2 free layout is (dt, ht, dcol)
            w2_np[s] = (
                W2n[g, e].astype(fp8).reshape(HT, P, DT, P)
                .transpose(1, 2, 0, 3).reshape(P, HT * D)
            )
            # bias scaled to match the WSCALE'd layer-1 products
            b1_v[:, s, :] = (b1n[g, e] * WSCALE).reshape(HT, P).T
        in_maps.append({"zt": zt_np, "w1": w1_np, "w2": w2_np, "b1": b1_np})

    # ---- 4. compile + run on the 8 NeuronCores
    _ensure_ntff_hook()
    from concourse.bass_utils import run_bass_kernel_spmd

    nc = _get_nc(Cs, has_b1=bool(np.any(b1n)))
    res = run_bass_kernel_spmd(
        nc, in_maps, core_ids=list(range(NCORES)),
        trace=bool(int(os.environ.get("KERNEL_TRACE", "0"))),
    )
    LAST_RESULTS = res

    # ---- 5. host combine (u comes back scaled by USCALE; fold 1/USCALE in)
    moe = np.zeros((G, N, D), np.float32)
    for c in range(NCORES):
        # u layout [p, dt*CT + c] -> u^T[d, c] -> [CT, D]
        u = (
            np.asarray(res.results[c]["u"], np.float32)
            .reshape(P, DT, CT).transpose(1, 0, 2).reshape(D, CT).T
        )
        for s in range(SLOTS):
            g, e = assign[(c, s)]
            toks = tok_lists[(g, e)]
            n = len(toks)
            w = scale_lists[(g, e)]
            contrib = u[offs[s]:offs[s] + n] * (w / USCALE)[:, None] \
                + w[:, None] * b2n[g, e][None, :]
            np.add.at(moe[g], toks, contrib)

    cpu = jax.devices("cpu")[0]
    with jax.default_device(cpu):
        zj = jnp.asarray(z)
        gi_j = jnp.asarray(gi)
        gsc_j = jnp.asarray(gsc)
        gw_dense = jnp.sum(
            jax.nn.one_hot(gi_j, G, dtype=jnp.float32) * gsc_j[..., None], axis=-2
        )  # [N, G]
        out = jnp.zeros((N, D), jnp.float32)
        gg = jnp.asarray(np.asarray(gln_g, np.float32))
        gb = jnp.asarray(np.asarray(gln_b, np.float32))
        for g in range(G):
            t = zj + jnp.asarray(moe[g])
            m = jnp.mean(t, axis=-1, keepdims=True)
            tc_ = t - m
            v = jnp.mean(tc_ * tc_, axis=-1, keepdims=True)
            y = tc_ * jax.lax.rsqrt(v + EPS) * gg[g] + gb[g]
            out = out + gw_dense[:, g:g + 1] * y
        result = np.asarray(out).reshape(B, T, D) + np.asarray(inp, np.float32)

    return result.astype(in_dtype)
